# revision 20
# baseline (speedup 1.0000x reference)
"""Head-sharded (tensor-parallel) CrossAttention kernel for 8 trn2 NeuronCores.

Problem shapes (hardcoded): B=2, L=2048, QD=1024, H=16, D=64.
Each core owns 2 heads end-to-end; the all-reduce over cores happens on host
(f32 partial sums).

v2 design — all matmuls in fp16 (1 PE cycle/row, no fp32r N>=256 constraint),
layouts chosen to keep every PE matmul at full 128-wide contraction where
possible and to keep the PE continuously busy (p-state ramp):

  P (projection), per bl-tile: x^T tiles (stationary) x Wqkv^T -> qkv PSUM
     [bl,384].  q/k staged to SBUF fp16 (ev/odd pairs pre-split via host-side
     weight-row permutation so RoPE runs on contiguous fp16 blocks with DVE
     2x); sum-of-squares on Pool+DVE; rsqrt via Newton on DVE; q normalized
     in place; RoPE on DVE; q^T/k^T via PE transposes (fp16) into the tail of
     the proj PSUM bank; v staged (fp16, with ones column) for the AV rhs.
     k's rms-norm is folded into the exp scale (per-lk-partition).
  A (attention), per (b, half=1024 lq, h) block: 16x scores^T [lk=128,1024]
     = 2 N=512 fp16 matmuls; ACT exp with scale=rrms_k/8 and bias=-10
     (softmax-invariant; keeps es in fp16 range) -> es_all[lk] fp16.
     AV runs one block behind (es_all double-buffered): per lq-tile,
     16 accumulating matmuls o[lq=128, 65] += es^T[lk-tile] @ v_aug
     (ones column -> denominator at col 64; one PSUM accumulation group per
     bank).  Normalize = DVE reciprocal + per-partition scalar multiply into
     o_both fp16.
  O (out projection), per bl-tile: o_both transposed on PE -> oT fp16;
     out[bl,1024] partial = oT^T @ Wproj^T in 2 N=512 matmuls; PSUM evacuated
     f32 (DVE/Pool alternating) and DMA'd out.

Emission interleaves P(b+1) tile-units and O(b) tile-units between A-block
lk-iterations so the PE queue never drains (stalls reset the PE p-state ramp).
"""

import numpy as np

import concourse.bass as bass
import concourse.tile as tile
from concourse import bacc, mybir
from concourse.bass_utils import run_bass_kernel_spmd
from concourse.masks import make_identity

F32 = mybir.dt.float32
F16 = mybir.dt.float16
AF = mybir.ActivationFunctionType
ALU = mybir.AluOpType

B, L, QD, H, D = 2, 2048, 1024, 16, 64
INNER = H * D
NCORES = 8
HL = H // NCORES          # heads per core = 2
M = HL * D                # 128 head-dim rows per core
BL = B * L                # 4096
NT = BL // 128            # 32 bl-tiles
NTH = NT // 2             # 16 tiles per batch
CT = QD // 128            # 8 contraction tiles
NLK = L // 128            # 16 lk tiles per batch
NSB = 2                   # P-phase tiles per sub-batch (newton batching;
                          # also bounds live proj-PSUM tiles to pP's 2 bufs)
EXP_BIAS = -10.0

_CACHE = {}


def _build_nc():
    nc = bacc.Bacc("TRN2", target_bir_lowering=False, debug=False)

    xt = nc.dram_tensor("xt", [NT, 128, CT, 128], F16, kind="ExternalInput")
    wqkv = nc.dram_tensor("wqkv", [128, CT, 3 * M], F16, kind="ExternalInput")
    wproj = nc.dram_tensor("wproj", [M, QD], F16, kind="ExternalInput")
    coefs = nc.dram_tensor("coefs", [NLK, 128, 4 * M], F16, kind="ExternalInput")
    outp = nc.dram_tensor("outp", [BL, QD], F32, kind="ExternalOutput")

    with tile.TileContext(nc) as tc:
        with (
            tc.tile_pool(name="res", bufs=1) as res,
            tc.tile_pool(name="xs", bufs=3) as xs,
            tc.tile_pool(name="cf", bufs=3) as cf,
            tc.tile_pool(name="stg", bufs=2) as stg,
            tc.tile_pool(name="wk", bufs=2) as wk,
            tc.tile_pool(name="esp", bufs=2) as esp,
            tc.tile_pool(name="nrm", bufs=2) as nrm,
            tc.tile_pool(name="obp", bufs=3) as obp,
            tc.tile_pool(name="osb", bufs=4) as osb,
            tc.tile_pool(name="pS", bufs=2, space="PSUM") as pS,   # scores 2x2 banks
            tc.tile_pool(name="pA", bufs=1, space="PSUM") as pA,   # o accum 1 bank
            tc.tile_pool(name="pP", bufs=2, space="PSUM") as pP,   # proj+transposes
            tc.tile_pool(name="pO", bufs=1, space="PSUM") as pO,   # out half
        ):
            # ---- residents ----
            wqkv_sb = res.tile([128, CT, 3 * M], F16)
            nc.sync.dma_start(out=wqkv_sb, in_=wqkv[:, :, :])
            wproj_sb = res.tile([M, QD], F16)
            nc.sync.dma_start(out=wproj_sb, in_=wproj[:, :])

            qhT = res.tile([M, BL], F16)      # rows: h0 d(64 perm), h1 d(64)
            khT = res.tile([M, BL], F16)
            vaug = res.tile([128, NT, HL, D + 1], F16)
            o_both = res.tile([128, B, NTH, M], F16)
            rr = res.tile([128, NT, 4], F32)

            nc.vector.memset(
                vaug[:, :, :, D : D + 1].rearrange("p a b c -> p (a b c)"), 1.0
            )
            ident_f = res.tile([128, 128], F32)
            make_identity(nc, ident_f)
            ident16 = res.tile([128, 128], F16)
            nc.vector.tensor_copy(ident16, ident_f)
            magic = res.tile([128, 16], mybir.dt.int32)
            nc.vector.memset(magic, 0x5F3759DF)
            biasap = res.tile([128, 1], F32)
            nc.vector.memset(biasap, EXP_BIAS)

            # ---------------- P phase units ----------------
            def p_unit_a(bb, sb, t, qk_st, ssq):
                """dma + proj matmuls + staging + ssq for one bl-tile."""
                jj = sb * NSB + t
                j = bb * NTH + jj
                xt_t = xs.tile([128, CT, 128], F16, tag="xt")
                nc.sync.dma_start(out=xt_t, in_=xt[j, :, :, :])
                ps = pP.tile([128, 512], F32, tag="pp")
                for ci in range(CT):
                    nc.tensor.matmul(
                        ps[:, 0 : 3 * M],
                        lhsT=xt_t[:, ci, :],
                        rhs=wqkv_sb[:, ci, :],
                        start=(ci == 0),
                        stop=(ci == CT - 1),
                    )
                # stage q|k and v (fp16) on DVE (gpsimd cannot touch PSUM)
                nc.vector.tensor_copy(qk_st[:, t, :], ps[:, 0 : 2 * M])
                nc.vector.tensor_copy(
                    vaug[:, j : j + 1, :, 0:D],
                    ps[:, 2 * M : 3 * M].rearrange(
                        "p (one h d) -> p one h d", one=1, h=HL
                    ),
                )
                # sum of squares (from staged fp16): Pool squares + reduce
                sqs = wk.tile([128, 2 * M], F32, tag="sqs")
                nc.gpsimd.tensor_mul(sqs, qk_st[:, t, :], qk_st[:, t, :])
                nc.vector.reduce_sum(
                    out=ssq[:, t, :].rearrange("p (a b) -> p a b", b=1),
                    in_=sqs.rearrange("p (a b) -> p a b", a=4),
                    axis=mybir.AxisListType.X,
                )

            def p_newton(bb, sb, ssq):
                """rrms via Newton rsqrt on DVE -> rr[:, j0:j0+NSB, :].
                cols 0:2 = rrms_q (per head), cols 2:4 = rrms_k/8."""
                j0 = bb * NTH + sb * NSB
                rrs = rr[:, j0 : j0 + NSB, :]
                nx = wk.tile([128, NSB, 4], F32, tag="nx")
                nc.vector.tensor_scalar(
                    out=nx[:, :, 0:2], in0=ssq[:, :, 0:2],
                    scalar1=1.0 / D, scalar2=1e-6, op0=ALU.mult, op1=ALU.add,
                )
                nc.vector.tensor_scalar(
                    out=nx[:, :, 2:4], in0=ssq[:, :, 2:4],
                    scalar1=1.0, scalar2=float(D) * 1e-6, op0=ALU.mult, op1=ALU.add,
                )
                sh = wk.tile([128, NSB, 4], mybir.dt.int32, tag="nsh")
                nc.vector.tensor_scalar(
                    out=sh, in0=nx.bitcast(mybir.dt.int32), scalar1=1,
                    scalar2=None, op0=ALU.logical_shift_right,
                )
                nc.vector.tensor_tensor(
                    out=rrs.bitcast(mybir.dt.int32),
                    in0=magic[:, 0 : NSB * 4].rearrange("p (a b) -> p a b", b=4),
                    in1=sh,
                    op=ALU.subtract,
                )
                ht = wk.tile([128, NSB, 4], F32, tag="nht")
                for _ in range(2):  # y *= 1.5 - 0.5*x*y*y
                    nc.vector.tensor_mul(ht, nx, rrs)
                    nc.vector.tensor_mul(ht, ht, rrs)
                    nc.vector.tensor_scalar(
                        out=ht, in0=ht, scalar1=-0.5, scalar2=1.5,
                        op0=ALU.mult, op1=ALU.add,
                    )
                    nc.vector.tensor_mul(rrs, rrs, ht)

            def p_unit_b(bb, sb, t, qk_st):
                """norm q, rope, transposes for one bl-tile."""
                jj = sb * NSB + t
                j = bb * NTH + jj
                for g in range(2):  # normalize q in place (per-head rrms_q)
                    nc.gpsimd.tensor_scalar_mul(
                        qk_st[:, t, g * D : (g + 1) * D],
                        qk_st[:, t, g * D : (g + 1) * D],
                        rr[:, j, g : g + 1],
                    )
                cft = cf.tile([128, 4 * M], F16, tag="cf")
                nc.sync.dma_start(out=cft, in_=coefs[jj, :, :])

                qk16 = wk.tile([128, 2 * M], F16, tag="ropeout", bufs=4)
                t1 = wk.tile([128, M], F16, tag="ropetmp")
                # views: [p, group(q0,q1,k0,k1), parity(ev|od block), d2]
                src = qk_st[:, t, :].rearrange(
                    "p (g two d2) -> p g two d2", g=4, two=2
                )
                dst = qk16.rearrange("p (g two d2) -> p g two d2", g=4, two=2)
                pl = [
                    cft[:, i * M : (i + 1) * M].rearrange(
                        "p (g d2) -> p g d2", g=4
                    )
                    for i in range(4)
                ]
                t1v = t1.rearrange("p (g d2) -> p g d2", g=4)
                # q groups (0:2) on DVE, k groups (2:4) on Pool — independent
                # chains so the engines don't serialize on each other.
                for eng, g0, g1 in ((nc.vector, 0, 2), (nc.gpsimd, 2, 4)):
                    ev = src[:, g0:g1, 0, :]
                    od = src[:, g0:g1, 1, :]
                    tv = t1v[:, g0:g1, :]
                    eng.tensor_mul(dst[:, g0:g1, 0, :], ev, pl[0][:, g0:g1, :])
                    eng.tensor_mul(tv, od, pl[1][:, g0:g1, :])
                    eng.tensor_add(dst[:, g0:g1, 0, :], dst[:, g0:g1, 0, :], tv)
                    eng.tensor_mul(dst[:, g0:g1, 1, :], ev, pl[2][:, g0:g1, :])
                    eng.tensor_mul(tv, od, pl[3][:, g0:g1, :])
                    eng.tensor_add(dst[:, g0:g1, 1, :], dst[:, g0:g1, 1, :], tv)

                # transposes via DMA XBAR (ACT hwdge queue; SP carries I/O)
                nc.scalar.dma_start(
                    out=qhT[:, j * 128 : (j + 1) * 128], in_=qk16[:, 0:M],
                    transpose=True,
                )
                nc.scalar.dma_start(
                    out=khT[:, j * 128 : (j + 1) * 128], in_=qk16[:, M : 2 * M],
                    transpose=True,
                )

            def gen_P(bb, sb_lo=0, sb_hi=None):
                """Yield emission units for P(bb) sub-batches [sb_lo, sb_hi)."""
                if sb_hi is None:
                    sb_hi = NTH // NSB
                for sb in range(sb_lo, sb_hi):
                    qk_st = stg.tile(
                        [128, NSB, 2 * M], F16, tag="stage", name="qk_st"
                    )
                    ssq = stg.tile([128, NSB, 4], F32, tag="ssq", name="ssq")
                    for t in range(NSB):
                        yield lambda bb=bb, sb=sb, t=t, q=qk_st, s=ssq: p_unit_a(
                            bb, sb, t, q, s
                        )
                    yield lambda bb=bb, sb=sb, s=ssq: p_newton(bb, sb, s)
                    for t in range(NSB):
                        yield lambda bb=bb, sb=sb, t=t, q=qk_st: p_unit_b(
                            bb, sb, t, q
                        )

            # ---------------- A phase ----------------
            def emit_scores_exp(bb, half, h, lk, es_cur):
                j = bb * NTH + lk
                pss = pS.tile([128, 1024], F32, tag="sc", name="pss")
                for h2 in range(2):
                    nc.tensor.matmul(
                        pss[:, h2 * 512 : (h2 + 1) * 512],
                        lhsT=khT[
                            h * D : (h + 1) * D,
                            bb * L + lk * 128 : bb * L + (lk + 1) * 128,
                        ],
                        rhs=qhT[
                            h * D : (h + 1) * D,
                            bb * L + half * 1024 + h2 * 512 :
                            bb * L + half * 1024 + (h2 + 1) * 512,
                        ],
                        start=True,
                        stop=True,
                    )
                nc.scalar.activation(
                    out=es_cur[:, lk, :], in_=pss, func=AF.Exp,
                    scale=rr[:, j, 2 + h : 3 + h], bias=biasap[:, 0:1],
                )

            def emit_av_tile(bb, half, h, i, es_prev):
                """AV for lq-tile i (0..7 within half) of the PREVIOUS block."""
                po = pA.tile([128, 128], F32, tag="oacc", name="po")
                for lk in range(NLK):
                    nc.tensor.matmul(
                        po[:, 0 : D + 1],
                        lhsT=es_prev[:, lk, i * 128 : (i + 1) * 128],
                        rhs=vaug[:, bb * NTH + lk, h, :],
                        start=(lk == 0),
                        stop=(lk == NLK - 1),
                        skip_group_check=True,
                    )
                rd = nrm.tile([128, 1], F32, tag="rd")
                nc.vector.reciprocal(rd, po[:, D : D + 1])
                ti = half * 8 + i
                nc.vector.tensor_scalar(
                    out=o_both[:, bb, ti, h * D : (h + 1) * D],
                    in0=po[:, 0:D],
                    scalar1=rd[:, 0:1], scalar2=None, op0=ALU.mult,
                )

            # ---------------- O phase units ----------------
            def o_unit(bb, ti, eng_flip):
                """transpose + out-proj + evac + dma for one bl-tile."""
                j = bb * NTH + ti
                oT = obp.tile([128, 128], F16, tag="oT")
                nc.sync.dma_start(out=oT, in_=o_both[:, bb, ti, :], transpose=True)
                for eo in range(2):
                    pout = pO.tile([128, 512], F32, tag="po2", name="pout")
                    nc.tensor.matmul(
                        pout, lhsT=oT, rhs=wproj_sb[:, eo * 512 : (eo + 1) * 512],
                        start=True, stop=True,
                    )
                    ob = osb.tile([128, 512], F32, tag="ob")
                    nc.vector.tensor_copy(ob, pout)
                    nc.sync.dma_start(
                        out=outp[j * 128 : (j + 1) * 128, eo * 512 : (eo + 1) * 512],
                        in_=ob,
                    )

            # ---------------- main schedule ----------------
            pending = []          # deque of emission units (P and O work)
            pend_idx = [0]

            def drain(n):
                k = 0
                while k < n and pend_idx[0] < len(pending):
                    pending[pend_idx[0]]()
                    pend_idx[0] += 1
                    k += 1

            # prologue: first half of P(0); rest interleaves into block 0
            for u in gen_P(0, 0, NTH // NSB // 2):
                u()
            pending.extend(gen_P(0, NTH // NSB // 2))

            blocks = [(bb, half, h) for bb in range(B) for half in range(2)
                      for h in range(HL)]
            prev_block = None
            prev_es = None
            es_tiles = {}
            o_ready = []          # (bb, half) chunks whose o_both is complete

            for bi, (bb, half, h) in enumerate(blocks):
                es_cur = esp.tile([128, NLK, 1024], F16, tag="es", name="es_all")
                # queue P(b+1) during b's first two blocks
                if bi == 0:
                    pending.extend(gen_P(1))
                for lk in range(NLK):
                    emit_scores_exp(bb, half, h, lk, es_cur)
                    # AV of the previous block: one lq-tile per 2 lk slots
                    if prev_block is not None and lk % 2 == 0:
                        pb, ph_alf, ph = prev_block
                        emit_av_tile(pb, ph_alf, ph, lk // 2, prev_es)
                        if lk == 14 and ph == 1:
                            o_ready.append((pb, ph_alf))
                    # interleave pending P/O units
                    if lk % 2 == 1:
                        drain(2)
                    elif len(pending) - pend_idx[0] > 32:
                        drain(2)
                # queue O work for completed halves
                while o_ready:
                    ob_b, ob_half = o_ready.pop(0)
                    for ti in range(ob_half * 8, ob_half * 8 + 8):
                        pending.append(
                            lambda bb2=ob_b, ti=ti, f=ti % 2: o_unit(bb2, ti, f)
                        )
                prev_block = (bb, half, h)
                prev_es = es_cur

            # tail: AV of the last block + remaining O work
            pb, ph_alf, ph = prev_block
            for i in range(8):
                emit_av_tile(pb, ph_alf, ph, i, prev_es)
            o_ready.append((pb, ph_alf))
            while o_ready:
                ob_b, ob_half = o_ready.pop(0)
                for ti in range(ob_half * 8, ob_half * 8 + 8):
                    pending.append(
                        lambda bb2=ob_b, ti=ti, f=ti % 2: o_unit(bb2, ti, f)
                    )
            drain(len(pending))

    nc.compile()
    return nc


_PERM = np.concatenate([np.arange(0, D, 2), np.arange(1, D, 2)])  # ev|od split


def _prep_inputs(x, pe, Wq, Wkv, Wproj, q_scale, k_scale):
    x = np.asarray(x, np.float32)
    xT = np.ascontiguousarray(x.reshape(BL, QD).T)                    # [QD, BL]
    xtt = np.ascontiguousarray(
        xT.reshape(CT, 128, NT, 128).transpose(2, 1, 0, 3)
    ).astype(np.float16)                                              # [NT, p, CT, n]

    pe = np.asarray(pe, np.float32)[0, 0]                             # [L, 32, 2, 2]
    qs, ks = np.asarray(q_scale, np.float32), np.asarray(k_scale, np.float32)

    def planes(scale):
        se, so = scale[0::2], scale[1::2]
        return (
            pe[:, :, 0, 0] * se[None, :],
            pe[:, :, 0, 1] * so[None, :],
            pe[:, :, 1, 0] * se[None, :],
            pe[:, :, 1, 1] * so[None, :],
        )

    pq, pk = planes(qs), planes(ks)
    coefs = np.empty((L, 4, 4, 32), np.float32)                       # [l, plane, grp, d2]
    for p_i in range(4):
        coefs[:, p_i, 0] = pq[p_i]
        coefs[:, p_i, 1] = pq[p_i]
        coefs[:, p_i, 2] = pk[p_i]
        coefs[:, p_i, 3] = pk[p_i]
    coefs = np.ascontiguousarray(coefs.reshape(NLK, 128, 4 * M)).astype(np.float16)

    Wq = np.asarray(Wq, np.float32)
    Wkv = np.asarray(Wkv, np.float32)
    Wproj = np.asarray(Wproj, np.float32)
    Wk_full, Wv_full = Wkv[:INNER], Wkv[INNER:]

    in_maps = []
    for c in range(NCORES):
        r0, r1 = c * M, (c + 1) * M
        wq_c, wk_c, wv_c = Wq[r0:r1], Wk_full[r0:r1], Wv_full[r0:r1]
        # permute q/k output dims to ev|od blocks per head (v stays in order)
        perm2 = np.concatenate([_PERM, _PERM + D])
        wq_c = wq_c[perm2]
        wk_c = wk_c[perm2]
        wqkv_c = np.concatenate([wq_c, wk_c, wv_c], axis=0)
        wqkv_t = np.ascontiguousarray(
            wqkv_c.T.reshape(CT, 128, 3 * M).transpose(1, 0, 2)
        ).astype(np.float16)                                          # [128, CT, 3M]
        wproj_c = np.ascontiguousarray(Wproj[:, r0:r1].T).astype(np.float16)
        in_maps.append(
            {"xt": xtt, "wqkv": wqkv_t, "wproj": wproj_c, "coefs": coefs}
        )
    return in_maps


def kernel(x, pe, Wq, Wkv, Wproj, bproj, q_scale, k_scale):
    if "nc" not in _CACHE:
        _CACHE["nc"] = _build_nc()
    nc = _CACHE["nc"]
    in_maps = _prep_inputs(x, pe, Wq, Wkv, Wproj, q_scale, k_scale)
    res = run_bass_kernel_spmd(nc, in_maps, core_ids=list(range(NCORES)))
    acc = np.zeros((BL, QD), np.float32)
    for c in range(NCORES):
        acc += np.asarray(res.results[c]["outp"], np.float32)
    acc += np.asarray(bproj, np.float32)[None, :]
    return acc.reshape(B, L, QD)


# revision 25
# speedup vs baseline: 1.6595x; 1.6595x over previous
"""Head-sharded (tensor-parallel) CrossAttention kernel for 8 trn2 NeuronCores.

Problem shapes (hardcoded): B=2, L=2048, QD=1024, H=16, D=64.
Each core owns 2 heads end-to-end; the all-reduce over cores happens on host
(f32 partial sums).

v2 design — all matmuls in fp16 (1 PE cycle/row, no fp32r N>=256 constraint),
layouts chosen to keep every PE matmul at full 128-wide contraction where
possible and to keep the PE continuously busy (p-state ramp):

  P (projection), per bl-tile: x^T tiles (stationary) x Wqkv^T -> qkv PSUM
     [bl,384].  q/k staged to SBUF fp16 (ev/odd pairs pre-split via host-side
     weight-row permutation so RoPE runs on contiguous fp16 blocks with DVE
     2x); sum-of-squares on Pool+DVE; rsqrt via Newton on DVE; q normalized
     in place; RoPE on DVE; q^T/k^T via PE transposes (fp16) into the tail of
     the proj PSUM bank; v staged (fp16, with ones column) for the AV rhs.
     k's rms-norm is folded into the exp scale (per-lk-partition).
  A (attention), per (b, half=1024 lq, h) block: 16x scores^T [lk=128,1024]
     = 2 N=512 fp16 matmuls; ACT exp with scale=rrms_k/8 and bias=-10
     (softmax-invariant; keeps es in fp16 range) -> es_all[lk] fp16.
     AV runs one block behind (es_all double-buffered): per lq-tile,
     16 accumulating matmuls o[lq=128, 65] += es^T[lk-tile] @ v_aug
     (ones column -> denominator at col 64; one PSUM accumulation group per
     bank).  Normalize = DVE reciprocal + per-partition scalar multiply into
     o_both fp16.
  O (out projection), per bl-tile: o_both transposed on PE -> oT fp16;
     out[bl,1024] partial = oT^T @ Wproj^T in 2 N=512 matmuls; PSUM evacuated
     f32 (DVE/Pool alternating) and DMA'd out.

Emission interleaves P(b+1) tile-units and O(b) tile-units between A-block
lk-iterations so the PE queue never drains (stalls reset the PE p-state ramp).
"""

import numpy as np

import concourse.bass as bass
import concourse.tile as tile
from concourse import bacc, mybir
from concourse.bass_utils import run_bass_kernel_spmd
from concourse.masks import make_identity

F32 = mybir.dt.float32
F16 = mybir.dt.float16
AF = mybir.ActivationFunctionType
ALU = mybir.AluOpType

B, L, QD, H, D = 2, 2048, 1024, 16, 64
INNER = H * D
NCORES = 8
HL = H // NCORES          # heads per core = 2
M = HL * D                # 128 head-dim rows per core
BL = B * L                # 4096
NT = BL // 128            # 32 bl-tiles
NTH = NT // 2             # 16 tiles per batch
CT = QD // 128            # 8 contraction tiles
NLK = L // 128            # 16 lk tiles per batch
NSB = 2                   # P-phase tiles per sub-batch (newton batching;
                          # also bounds live proj-PSUM tiles to pP's 2 bufs)
EXP_BIAS = -10.0

_CACHE = {}


def _build_nc():
    nc = bacc.Bacc("TRN2", target_bir_lowering=False, debug=False)

    xt = nc.dram_tensor("xt", [NT, 128, CT, 128], F16, kind="ExternalInput")
    wqkv = nc.dram_tensor("wqkv", [128, CT, 3 * M], F16, kind="ExternalInput")
    wproj = nc.dram_tensor("wproj", [M, QD], F16, kind="ExternalInput")
    coefs = nc.dram_tensor("coefs", [NLK, 128, 4 * M], F16, kind="ExternalInput")
    outp = nc.dram_tensor("outp", [BL, QD], F32, kind="ExternalOutput")

    with tile.TileContext(nc) as tc:
        with (
            tc.tile_pool(name="res", bufs=1) as res,
            tc.tile_pool(name="xs", bufs=3) as xs,
            tc.tile_pool(name="cf", bufs=3) as cf,
            tc.tile_pool(name="stg", bufs=2) as stg,
            tc.tile_pool(name="wk", bufs=2) as wk,
            tc.tile_pool(name="esp", bufs=2) as esp,
            tc.tile_pool(name="nrm", bufs=2) as nrm,
            tc.tile_pool(name="obp", bufs=3) as obp,
            tc.tile_pool(name="osb", bufs=4) as osb,
            tc.tile_pool(name="pS", bufs=2, space="PSUM") as pS,   # scores 2x2 banks
            tc.tile_pool(name="pA", bufs=1, space="PSUM") as pA,   # o accum 1 bank
            tc.tile_pool(name="pP", bufs=2, space="PSUM") as pP,   # proj+transposes
            tc.tile_pool(name="pO", bufs=1, space="PSUM") as pO,   # out half
        ):
            # ---- residents ----
            wqkv_sb = res.tile([128, CT, 3 * M], F16)
            nc.sync.dma_start(out=wqkv_sb, in_=wqkv[:, :, :])
            wproj_sb = res.tile([M, QD], F16)
            nc.sync.dma_start(out=wproj_sb, in_=wproj[:, :])

            qhT = res.tile([M, BL], F16)      # rows: h0 d(64 perm), h1 d(64)
            khT = res.tile([M, BL], F16)
            vaug = res.tile([128, NT, HL, D + 1], F16)
            o_both = res.tile([128, B, NTH, M], F16)
            rr = res.tile([128, NT, 4], F32)

            nc.vector.memset(
                vaug[:, :, :, D : D + 1].rearrange("p a b c -> p (a b c)"), 1.0
            )
            ident_f = res.tile([128, 128], F32)
            make_identity(nc, ident_f)
            ident16 = res.tile([128, 128], F16)
            nc.vector.tensor_copy(ident16, ident_f)
            magic = res.tile([128, 16], mybir.dt.int32)
            nc.vector.memset(magic, 0x5F3759DF)
            biasap = res.tile([128, 1], F32)
            nc.vector.memset(biasap, EXP_BIAS)

            # ---------------- P phase units ----------------
            def p_unit_a(bb, sb, t, qk_st, ssq):
                """dma + proj matmuls + staging + ssq for one bl-tile."""
                jj = sb * NSB + t
                j = bb * NTH + jj
                xt_t = xs.tile([128, CT, 128], F16, tag="xt")
                nc.sync.dma_start(out=xt_t, in_=xt[j, :, :, :])
                ps = pP.tile([128, 512], F32, tag="pp")
                for ci in range(CT):
                    nc.tensor.matmul(
                        ps[:, 0 : 3 * M],
                        lhsT=xt_t[:, ci, :],
                        rhs=wqkv_sb[:, ci, :],
                        start=(ci == 0),
                        stop=(ci == CT - 1),
                    )
                # stage q|k and v (fp16) on DVE (gpsimd cannot touch PSUM)
                nc.vector.tensor_copy(qk_st[:, t, :], ps[:, 0 : 2 * M])
                nc.vector.tensor_copy(
                    vaug[:, j : j + 1, :, 0:D],
                    ps[:, 2 * M : 3 * M].rearrange(
                        "p (one h d) -> p one h d", one=1, h=HL
                    ),
                )
                # sum of squares (from staged fp16): Pool squares, DVE reduce
                sqs = wk.tile([128, 2 * M], F32, tag="sqs")
                nc.gpsimd.tensor_mul(sqs, qk_st[:, t, :], qk_st[:, t, :])
                nc.vector.reduce_sum(
                    out=ssq[:, t, :].rearrange("p (a b) -> p a b", b=1),
                    in_=sqs.rearrange("p (a b) -> p a b", a=4),
                    axis=mybir.AxisListType.X,
                )
                return ps

            def p_newton(bb, sb, ssq):
                """rrms via Newton rsqrt on DVE -> rr[:, j0:j0+NSB, :].
                cols 0:2 = rrms_q (per head), cols 2:4 = rrms_k/8."""
                j0 = bb * NTH + sb * NSB
                rrs = rr[:, j0 : j0 + NSB, :]
                nx = wk.tile([128, NSB, 4], F32, tag="nx")
                nc.vector.tensor_scalar(
                    out=nx[:, :, 0:2], in0=ssq[:, :, 0:2],
                    scalar1=1.0 / D, scalar2=1e-6, op0=ALU.mult, op1=ALU.add,
                )
                nc.vector.tensor_scalar(
                    out=nx[:, :, 2:4], in0=ssq[:, :, 2:4],
                    scalar1=1.0, scalar2=float(D) * 1e-6, op0=ALU.mult, op1=ALU.add,
                )
                sh = wk.tile([128, NSB, 4], mybir.dt.int32, tag="nsh")
                nc.vector.tensor_scalar(
                    out=sh, in0=nx.bitcast(mybir.dt.int32), scalar1=1,
                    scalar2=None, op0=ALU.logical_shift_right,
                )
                nc.vector.tensor_tensor(
                    out=rrs.bitcast(mybir.dt.int32),
                    in0=magic[:, 0 : NSB * 4].rearrange("p (a b) -> p a b", b=4),
                    in1=sh,
                    op=ALU.subtract,
                )
                ht = wk.tile([128, NSB, 4], F32, tag="nht")
                for _ in range(2):  # y *= 1.5 - 0.5*x*y*y
                    nc.vector.tensor_mul(ht, nx, rrs)
                    nc.vector.tensor_mul(ht, ht, rrs)
                    nc.vector.tensor_scalar(
                        out=ht, in0=ht, scalar1=-0.5, scalar2=1.5,
                        op0=ALU.mult, op1=ALU.add,
                    )
                    nc.vector.tensor_mul(rrs, rrs, ht)

            def p_unit_b(bb, sb, t, qk_st, ps):
                """norm q, rope, transposes for one bl-tile."""
                jj = sb * NSB + t
                j = bb * NTH + jj
                for g in range(2):  # normalize q in place (per-head rrms_q)
                    nc.gpsimd.tensor_scalar_mul(
                        qk_st[:, t, g * D : (g + 1) * D],
                        qk_st[:, t, g * D : (g + 1) * D],
                        rr[:, j, g : g + 1],
                    )
                cft = cf.tile([128, 4 * M], F16, tag="cf")
                nc.sync.dma_start(out=cft, in_=coefs[jj, :, :])

                qk16 = wk.tile([128, 2 * M], F16, tag="ropeout", bufs=4)
                t1 = wk.tile([128, M], F16, tag="ropetmp")
                # views: [p, group(q0,q1,k0,k1), parity(ev|od block), d2]
                src = qk_st[:, t, :].rearrange(
                    "p (g two d2) -> p g two d2", g=4, two=2
                )
                dst = qk16.rearrange("p (g two d2) -> p g two d2", g=4, two=2)
                pl = [
                    cft[:, i * M : (i + 1) * M].rearrange(
                        "p (g d2) -> p g d2", g=4
                    )
                    for i in range(4)
                ]
                t1v = t1.rearrange("p (g d2) -> p g d2", g=4)
                # q groups (0:2) on DVE, k groups (2:4) on Pool — independent
                # chains so the engines don't serialize on each other.
                for eng, g0, g1 in ((nc.vector, 0, 2), (nc.gpsimd, 2, 4)):
                    ev = src[:, g0:g1, 0, :]
                    od = src[:, g0:g1, 1, :]
                    tv = t1v[:, g0:g1, :]
                    eng.tensor_mul(dst[:, g0:g1, 0, :], ev, pl[0][:, g0:g1, :])
                    eng.tensor_mul(tv, od, pl[1][:, g0:g1, :])
                    eng.tensor_add(dst[:, g0:g1, 0, :], dst[:, g0:g1, 0, :], tv)
                    eng.tensor_mul(dst[:, g0:g1, 1, :], ev, pl[2][:, g0:g1, :])
                    eng.tensor_mul(tv, od, pl[3][:, g0:g1, :])
                    eng.tensor_add(dst[:, g0:g1, 1, :], dst[:, g0:g1, 1, :], tv)

                # transposes on PE into the tail of the proj bank (fp16 bitcast)
                tr = ps[:, 384:512].bitcast(F16)  # [128, 256]
                nc.tensor.transpose(tr[:, 0:128], qk16[:, 0:M], ident16)
                nc.tensor.transpose(tr[:, 128:256], qk16[:, M : 2 * M], ident16)
                nc.vector.tensor_copy(qhT[:, j * 128 : (j + 1) * 128], tr[:, 0:128])
                nc.vector.tensor_copy(khT[:, j * 128 : (j + 1) * 128], tr[:, 128:256])

            def gen_P(bb, sb_lo=0, sb_hi=None):
                """Yield emission units for P(bb) sub-batches [sb_lo, sb_hi)."""
                if sb_hi is None:
                    sb_hi = NTH // NSB
                for sb in range(sb_lo, sb_hi):
                    qk_st = stg.tile(
                        [128, NSB, 2 * M], F16, tag="stage", name="qk_st"
                    )
                    ssq = stg.tile([128, NSB, 4], F32, tag="ssq", name="ssq")
                    pss = []
                    for t in range(NSB):
                        yield lambda bb=bb, sb=sb, t=t, q=qk_st, s=ssq, ps=pss: ps.append(
                            p_unit_a(bb, sb, t, q, s)
                        )
                    yield lambda bb=bb, sb=sb, s=ssq: p_newton(bb, sb, s)
                    for t in range(NSB):
                        yield lambda bb=bb, sb=sb, t=t, q=qk_st, ps=pss: p_unit_b(
                            bb, sb, t, q, ps[t]
                        )

            # ---------------- A phase ----------------
            def emit_scores_exp(bb, half, h, lk, es_cur):
                j = bb * NTH + lk
                pss = pS.tile([128, 1024], F32, tag="sc", name="pss")
                for h2 in range(2):
                    nc.tensor.matmul(
                        pss[:, h2 * 512 : (h2 + 1) * 512],
                        lhsT=khT[
                            h * D : (h + 1) * D,
                            bb * L + lk * 128 : bb * L + (lk + 1) * 128,
                        ],
                        rhs=qhT[
                            h * D : (h + 1) * D,
                            bb * L + half * 1024 + h2 * 512 :
                            bb * L + half * 1024 + (h2 + 1) * 512,
                        ],
                        start=True,
                        stop=True,
                    )
                nc.scalar.activation(
                    out=es_cur[:, lk, :], in_=pss, func=AF.Exp,
                    scale=rr[:, j, 2 + h : 3 + h], bias=biasap[:, 0:1],
                )

            def emit_av_tile(bb, half, h, i, es_prev):
                """AV for lq-tile i (0..7 within half) of the PREVIOUS block."""
                po = pA.tile([128, 128], F32, tag="oacc", name="po")
                for lk in range(NLK):
                    nc.tensor.matmul(
                        po[:, 0 : D + 1],
                        lhsT=es_prev[:, lk, i * 128 : (i + 1) * 128],
                        rhs=vaug[:, bb * NTH + lk, h, :],
                        start=(lk == 0),
                        stop=(lk == NLK - 1),
                        skip_group_check=True,
                    )
                rd = nrm.tile([128, 1], F32, tag="rd")
                nc.vector.reciprocal(rd, po[:, D : D + 1])
                ti = half * 8 + i
                nc.vector.tensor_scalar(
                    out=o_both[:, bb, ti, h * D : (h + 1) * D],
                    in0=po[:, 0:D],
                    scalar1=rd[:, 0:1], scalar2=None, op0=ALU.mult,
                )

            # ---------------- O phase units ----------------
            def o_unit(bb, ti, eng_flip):
                """transpose + out-proj + evac + dma for one bl-tile."""
                j = bb * NTH + ti
                ps = pP.tile([128, 512], F32, tag="pp", name="ot_ps")
                tr = ps[:, 0:64].bitcast(F16)  # [128, 128]
                nc.tensor.transpose(tr, o_both[:, bb, ti, :], ident16)
                oT = obp.tile([128, 128], F16, tag="oT")
                nc.vector.tensor_copy(oT, tr)
                for eo in range(2):
                    pout = pO.tile([128, 512], F32, tag="po2", name="pout")
                    nc.tensor.matmul(
                        pout, lhsT=oT, rhs=wproj_sb[:, eo * 512 : (eo + 1) * 512],
                        start=True, stop=True,
                    )
                    ob = osb.tile([128, 512], F32, tag="ob")
                    nc.vector.tensor_copy(ob, pout)
                    nc.sync.dma_start(
                        out=outp[j * 128 : (j + 1) * 128, eo * 512 : (eo + 1) * 512],
                        in_=ob,
                    )

            # ---------------- main schedule ----------------
            pending = []          # deque of emission units (P and O work)
            pend_idx = [0]

            def drain(n):
                k = 0
                while k < n and pend_idx[0] < len(pending):
                    pending[pend_idx[0]]()
                    pend_idx[0] += 1
                    k += 1

            # prologue: first half of P(0); rest interleaves into block 0
            for u in gen_P(0, 0, NTH // NSB // 2):
                u()
            pending.extend(gen_P(0, NTH // NSB // 2))

            blocks = [(bb, half, h) for bb in range(B) for half in range(2)
                      for h in range(HL)]
            prev_block = None
            prev_es = None
            es_tiles = {}
            o_ready = []          # (bb, half) chunks whose o_both is complete

            for bi, (bb, half, h) in enumerate(blocks):
                es_cur = esp.tile([128, NLK, 1024], F16, tag="es", name="es_all")
                # queue P(b+1) during b's first two blocks
                if bi == 0:
                    pending.extend(gen_P(1))
                for lk in range(NLK):
                    emit_scores_exp(bb, half, h, lk, es_cur)
                    # AV of the previous block: one lq-tile per 2 lk slots
                    if prev_block is not None and lk % 2 == 0:
                        pb, ph_alf, ph = prev_block
                        emit_av_tile(pb, ph_alf, ph, lk // 2, prev_es)
                        if lk == 14 and ph == 1:
                            o_ready.append((pb, ph_alf))
                    # interleave pending P/O units
                    if lk % 2 == 1:
                        drain(2)
                    elif len(pending) - pend_idx[0] > 32:
                        drain(2)
                # queue O work for completed halves
                while o_ready:
                    ob_b, ob_half = o_ready.pop(0)
                    for ti in range(ob_half * 8, ob_half * 8 + 8):
                        pending.append(
                            lambda bb2=ob_b, ti=ti, f=ti % 2: o_unit(bb2, ti, f)
                        )
                prev_block = (bb, half, h)
                prev_es = es_cur

            # tail: AV of the last block + remaining O work
            pb, ph_alf, ph = prev_block
            for i in range(8):
                emit_av_tile(pb, ph_alf, ph, i, prev_es)
            o_ready.append((pb, ph_alf))
            while o_ready:
                ob_b, ob_half = o_ready.pop(0)
                for ti in range(ob_half * 8, ob_half * 8 + 8):
                    pending.append(
                        lambda bb2=ob_b, ti=ti, f=ti % 2: o_unit(bb2, ti, f)
                    )
            drain(len(pending))

    nc.compile()
    return nc


_PERM = np.concatenate([np.arange(0, D, 2), np.arange(1, D, 2)])  # ev|od split


def _prep_inputs(x, pe, Wq, Wkv, Wproj, q_scale, k_scale):
    x = np.asarray(x, np.float32)
    xT = np.ascontiguousarray(x.reshape(BL, QD).T)                    # [QD, BL]
    xtt = np.ascontiguousarray(
        xT.reshape(CT, 128, NT, 128).transpose(2, 1, 0, 3)
    ).astype(np.float16)                                              # [NT, p, CT, n]

    pe = np.asarray(pe, np.float32)[0, 0]                             # [L, 32, 2, 2]
    qs, ks = np.asarray(q_scale, np.float32), np.asarray(k_scale, np.float32)

    def planes(scale):
        se, so = scale[0::2], scale[1::2]
        return (
            pe[:, :, 0, 0] * se[None, :],
            pe[:, :, 0, 1] * so[None, :],
            pe[:, :, 1, 0] * se[None, :],
            pe[:, :, 1, 1] * so[None, :],
        )

    pq, pk = planes(qs), planes(ks)
    coefs = np.empty((L, 4, 4, 32), np.float32)                       # [l, plane, grp, d2]
    for p_i in range(4):
        coefs[:, p_i, 0] = pq[p_i]
        coefs[:, p_i, 1] = pq[p_i]
        coefs[:, p_i, 2] = pk[p_i]
        coefs[:, p_i, 3] = pk[p_i]
    coefs = np.ascontiguousarray(coefs.reshape(NLK, 128, 4 * M)).astype(np.float16)

    Wq = np.asarray(Wq, np.float32)
    Wkv = np.asarray(Wkv, np.float32)
    Wproj = np.asarray(Wproj, np.float32)
    Wk_full, Wv_full = Wkv[:INNER], Wkv[INNER:]

    in_maps = []
    for c in range(NCORES):
        r0, r1 = c * M, (c + 1) * M
        wq_c, wk_c, wv_c = Wq[r0:r1], Wk_full[r0:r1], Wv_full[r0:r1]
        # permute q/k output dims to ev|od blocks per head (v stays in order)
        perm2 = np.concatenate([_PERM, _PERM + D])
        wq_c = wq_c[perm2]
        wk_c = wk_c[perm2]
        wqkv_c = np.concatenate([wq_c, wk_c, wv_c], axis=0)
        wqkv_t = np.ascontiguousarray(
            wqkv_c.T.reshape(CT, 128, 3 * M).transpose(1, 0, 2)
        ).astype(np.float16)                                          # [128, CT, 3M]
        wproj_c = np.ascontiguousarray(Wproj[:, r0:r1].T).astype(np.float16)
        in_maps.append(
            {"xt": xtt, "wqkv": wqkv_t, "wproj": wproj_c, "coefs": coefs}
        )
    return in_maps


def kernel(x, pe, Wq, Wkv, Wproj, bproj, q_scale, k_scale):
    if "nc" not in _CACHE:
        _CACHE["nc"] = _build_nc()
    nc = _CACHE["nc"]
    in_maps = _prep_inputs(x, pe, Wq, Wkv, Wproj, q_scale, k_scale)
    res = run_bass_kernel_spmd(nc, in_maps, core_ids=list(range(NCORES)))
    acc = np.zeros((BL, QD), np.float32)
    for c in range(NCORES):
        acc += np.asarray(res.results[c]["outp"], np.float32)
    acc += np.asarray(bproj, np.float32)[None, :]
    return acc.reshape(B, L, QD)


# revision 32
# speedup vs baseline: 1.9362x; 1.1667x over previous
"""Head-sharded (tensor-parallel) CrossAttention kernel for 8 trn2 NeuronCores.

Problem shapes (hardcoded): B=2, L=2048, QD=1024, H=16, D=64.
Each core owns 2 heads end-to-end; the all-reduce over cores happens on host
(f32 partial sums).

v2 design — all matmuls in fp16 (1 PE cycle/row, no fp32r N>=256 constraint),
layouts chosen to keep every PE matmul at full 128-wide contraction where
possible and to keep the PE continuously busy (p-state ramp):

  P (projection), per bl-tile: x^T tiles (stationary) x Wqkv^T -> qkv PSUM
     [bl,384].  q/k staged to SBUF fp16 (ev/odd pairs pre-split via host-side
     weight-row permutation so RoPE runs on contiguous fp16 blocks with DVE
     2x); sum-of-squares on Pool+DVE; rsqrt via Newton on DVE; q normalized
     in place; RoPE on DVE; q^T/k^T via PE transposes (fp16) into the tail of
     the proj PSUM bank; v staged (fp16, with ones column) for the AV rhs.
     k's rms-norm is folded into the exp scale (per-lk-partition).
  A (attention), per (b, half=1024 lq, h) block: 16x scores^T [lk=128,1024]
     = 2 N=512 fp16 matmuls; ACT exp with scale=rrms_k/8 and bias=-10
     (softmax-invariant; keeps es in fp16 range) -> es_all[lk] fp16.
     AV runs one block behind (es_all double-buffered): per lq-tile,
     16 accumulating matmuls o[lq=128, 65] += es^T[lk-tile] @ v_aug
     (ones column -> denominator at col 64; one PSUM accumulation group per
     bank).  Normalize = DVE reciprocal + per-partition scalar multiply into
     o_both fp16.
  O (out projection), per bl-tile: o_both transposed on PE -> oT fp16;
     out[bl,1024] partial = oT^T @ Wproj^T in 2 N=512 matmuls; PSUM evacuated
     f32 (DVE/Pool alternating) and DMA'd out.

Emission interleaves P(b+1) tile-units and O(b) tile-units between A-block
lk-iterations so the PE queue never drains (stalls reset the PE p-state ramp).
"""

import numpy as np

import concourse.bass as bass
import concourse.tile as tile
from concourse import bacc, mybir
from concourse.bass_utils import run_bass_kernel_spmd
from concourse.masks import make_identity

F32 = mybir.dt.float32
F16 = mybir.dt.float16
AF = mybir.ActivationFunctionType
ALU = mybir.AluOpType

B, L, QD, H, D = 2, 2048, 1024, 16, 64
INNER = H * D
NCORES = 8
HL = H // NCORES          # heads per core = 2
M = HL * D                # 128 head-dim rows per core
BL = B * L                # 4096
NT = BL // 128            # 32 bl-tiles
NTH = NT // 2             # 16 tiles per batch
CT = QD // 128            # 8 contraction tiles
NLK = L // 128            # 16 lk tiles per batch
NSB = 4                   # P-phase tiles per sub-batch (newton batching)
EXP_BIAS = -10.0

_CACHE = {}


def _build_nc():
    nc = bacc.Bacc("TRN2", target_bir_lowering=False, debug=False)

    xt = nc.dram_tensor("xt", [NT, 128, CT, 128], F16, kind="ExternalInput")
    wqkv = nc.dram_tensor("wqkv", [128, CT, 3 * M], F16, kind="ExternalInput")
    wproj = nc.dram_tensor("wproj", [M, QD], F16, kind="ExternalInput")
    coefs = nc.dram_tensor("coefs", [NLK, 128, 4 * M], F16, kind="ExternalInput")
    outp = nc.dram_tensor("outp", [BL, QD], F32, kind="ExternalOutput")

    with tile.TileContext(nc) as tc:
        with (
            tc.tile_pool(name="res", bufs=1) as res,
            tc.tile_pool(name="xs", bufs=3) as xs,
            tc.tile_pool(name="cf", bufs=3) as cf,
            tc.tile_pool(name="stg", bufs=2) as stg,
            tc.tile_pool(name="wk", bufs=2) as wk,
            tc.tile_pool(name="esp", bufs=2) as esp,
            tc.tile_pool(name="nrm", bufs=2) as nrm,
            tc.tile_pool(name="obp", bufs=3) as obp,
            tc.tile_pool(name="osb", bufs=4) as osb,
            tc.tile_pool(name="pS", bufs=2, space="PSUM") as pS,   # scores 2x2 banks
            tc.tile_pool(name="pA", bufs=1, space="PSUM") as pA,   # o accum 1 bank
            tc.tile_pool(name="pP", bufs=2, space="PSUM") as pP,   # proj+transposes
            tc.tile_pool(name="pO", bufs=1, space="PSUM") as pO,   # out half
        ):
            # ---- residents ----
            wqkv_sb = res.tile([128, CT, 3 * M], F16)
            nc.sync.dma_start(out=wqkv_sb, in_=wqkv[:, :, :])
            wproj_sb = res.tile([M, QD], F16)
            nc.sync.dma_start(out=wproj_sb, in_=wproj[:, :])

            qhT = res.tile([M, BL], F16)      # rows: h0 d(64 perm), h1 d(64)
            khT = res.tile([M, BL], F16)
            vaug = res.tile([128, NT, HL, D + 1], F16)
            o_both = res.tile([128, B, NTH, M], F16)
            rr = res.tile([128, NT, 4], F32)

            nc.vector.memset(
                vaug[:, :, :, D : D + 1].rearrange("p a b c -> p (a b c)"), 1.0
            )
            ident_f = res.tile([128, 128], F32)
            make_identity(nc, ident_f)
            ident16 = res.tile([128, 128], F16)
            nc.vector.tensor_copy(ident16, ident_f)
            magic = res.tile([128, 16], mybir.dt.int32)
            nc.vector.memset(magic, 0x5F3759DF)
            biasap = res.tile([128, 1], F32)
            nc.vector.memset(biasap, EXP_BIAS)

            # ---------------- P phase units ----------------
            def p_unit_a(bb, sb, t, qk_st, ssq):
                """dma + proj matmuls + staging + ssq for one bl-tile."""
                jj = sb * NSB + t
                j = bb * NTH + jj
                xt_t = xs.tile([128, CT, 128], F16, tag="xt")
                nc.sync.dma_start(out=xt_t, in_=xt[j, :, :, :])
                ps = pP.tile([128, 512], F32, tag="pp")
                for ci in range(CT):
                    nc.tensor.matmul(
                        ps[:, 0 : 3 * M],
                        lhsT=xt_t[:, ci, :],
                        rhs=wqkv_sb[:, ci, :],
                        start=(ci == 0),
                        stop=(ci == CT - 1),
                    )
                # stage q|k and v (fp16) on DVE (gpsimd cannot touch PSUM)
                nc.vector.tensor_copy(qk_st[:, t, :], ps[:, 0 : 2 * M])
                nc.vector.tensor_copy(
                    vaug[:, j : j + 1, :, 0:D],
                    ps[:, 2 * M : 3 * M].rearrange(
                        "p (one h d) -> p one h d", one=1, h=HL
                    ),
                )
                # sum of squares (from staged fp16): Pool squares, DVE reduce
                sqs = wk.tile([128, 2 * M], F32, tag="sqs")
                nc.gpsimd.tensor_mul(sqs, qk_st[:, t, :], qk_st[:, t, :])
                nc.vector.reduce_sum(
                    out=ssq[:, t, :].rearrange("p (a b) -> p a b", b=1),
                    in_=sqs.rearrange("p (a b) -> p a b", a=4),
                    axis=mybir.AxisListType.X,
                )
                return ps

            def p_newton(bb, sb, ssq):
                """rrms via Newton rsqrt on DVE -> rr[:, j0:j0+NSB, :].
                cols 0:2 = rrms_q (per head), cols 2:4 = rrms_k/8."""
                j0 = bb * NTH + sb * NSB
                rrs = rr[:, j0 : j0 + NSB, :]
                nx = wk.tile([128, NSB, 4], F32, tag="nx")
                nc.vector.tensor_scalar(
                    out=nx[:, :, 0:2], in0=ssq[:, :, 0:2],
                    scalar1=1.0 / D, scalar2=1e-6, op0=ALU.mult, op1=ALU.add,
                )
                nc.vector.tensor_scalar(
                    out=nx[:, :, 2:4], in0=ssq[:, :, 2:4],
                    scalar1=1.0, scalar2=float(D) * 1e-6, op0=ALU.mult, op1=ALU.add,
                )
                sh = wk.tile([128, NSB, 4], mybir.dt.int32, tag="nsh")
                nc.vector.tensor_scalar(
                    out=sh, in0=nx.bitcast(mybir.dt.int32), scalar1=1,
                    scalar2=None, op0=ALU.logical_shift_right,
                )
                nc.vector.tensor_tensor(
                    out=rrs.bitcast(mybir.dt.int32),
                    in0=magic[:, 0 : NSB * 4].rearrange("p (a b) -> p a b", b=4),
                    in1=sh,
                    op=ALU.subtract,
                )
                ht = wk.tile([128, NSB, 4], F32, tag="nht")
                for _ in range(2):  # y *= 1.5 - 0.5*x*y*y
                    nc.vector.tensor_mul(ht, nx, rrs)
                    nc.vector.tensor_mul(ht, ht, rrs)
                    nc.vector.tensor_scalar(
                        out=ht, in0=ht, scalar1=-0.5, scalar2=1.5,
                        op0=ALU.mult, op1=ALU.add,
                    )
                    nc.vector.tensor_mul(rrs, rrs, ht)

            def p_unit_b(bb, sb, t, qk_st, qk16s):
                """norm q + rope for one bl-tile (DVE/Pool only, no PE)."""
                jj = sb * NSB + t
                j = bb * NTH + jj
                for g in range(2):  # normalize q in place (per-head rrms_q)
                    nc.vector.tensor_scalar(
                        out=qk_st[:, t, g * D : (g + 1) * D],
                        in0=qk_st[:, t, g * D : (g + 1) * D],
                        scalar1=rr[:, j, g : g + 1], scalar2=None, op0=ALU.mult,
                    )
                cft = cf.tile([128, 4 * M], F16, tag="cf")
                nc.sync.dma_start(out=cft, in_=coefs[jj, :, :])

                qk16 = wk.tile([128, 2 * M], F16, tag="ropeout", bufs=6)
                qk16s.append(qk16)
                t1 = wk.tile([128, M], F16, tag="ropetmp")
                # views: [p, group(q0,q1,k0,k1), parity(ev|od block), d2]
                src = qk_st[:, t, :].rearrange(
                    "p (g two d2) -> p g two d2", g=4, two=2
                )
                dst = qk16.rearrange("p (g two d2) -> p g two d2", g=4, two=2)
                pl = [
                    cft[:, i * M : (i + 1) * M].rearrange(
                        "p (g d2) -> p g d2", g=4
                    )
                    for i in range(4)
                ]
                t1v = t1.rearrange("p (g d2) -> p g d2", g=4)
                # q groups (0:2) on DVE, k groups (2:4) on Pool — independent
                # chains so the engines don't serialize on each other.
                for eng, g0, g1 in ((nc.vector, 0, 2), (nc.gpsimd, 2, 4)):
                    ev = src[:, g0:g1, 0, :]
                    od = src[:, g0:g1, 1, :]
                    tv = t1v[:, g0:g1, :]
                    eng.tensor_mul(dst[:, g0:g1, 0, :], ev, pl[0][:, g0:g1, :])
                    eng.tensor_mul(tv, od, pl[1][:, g0:g1, :])
                    eng.tensor_add(dst[:, g0:g1, 0, :], dst[:, g0:g1, 0, :], tv)
                    eng.tensor_mul(dst[:, g0:g1, 1, :], ev, pl[2][:, g0:g1, :])
                    eng.tensor_mul(tv, od, pl[3][:, g0:g1, :])
                    eng.tensor_add(dst[:, g0:g1, 1, :], dst[:, g0:g1, 1, :], tv)

            def p_unit_c(bb, sb, t, qk16):
                """PE transposes of rope output into a fresh pP rotation slot.
                Emitted several slots after p_unit_b so the DVE/Pool rope
                chain is finished before the PE reaches these (in-order PE
                queue: a waiting transpose blocks everything behind it)."""
                j = bb * NTH + sb * NSB + t
                ps = pP.tile([128, 512], F32, tag="pp", name="tr_ps")
                tr = ps[:, 384:512].bitcast(F16)  # [128, 256]
                nc.tensor.transpose(tr[:, 0:128], qk16[:, 0:M], ident16)
                nc.tensor.transpose(tr[:, 128:256], qk16[:, M : 2 * M], ident16)
                nc.vector.tensor_copy(qhT[:, j * 128 : (j + 1) * 128], tr[:, 0:128])
                nc.vector.tensor_copy(khT[:, j * 128 : (j + 1) * 128], tr[:, 128:256])

            def gen_P(bb, sb_lo=0, sb_hi=None):
                """Yield emission units for P(bb) sub-batches [sb_lo, sb_hi)."""
                if sb_hi is None:
                    sb_hi = NTH // NSB
                for sb in range(sb_lo, sb_hi):
                    qk_st = stg.tile(
                        [128, NSB, 2 * M], F16, tag="stage", name="qk_st"
                    )
                    ssq = stg.tile([128, NSB, 4], F32, tag="ssq", name="ssq")
                    qk16s = []
                    for t in range(NSB):
                        yield lambda bb=bb, sb=sb, t=t, q=qk_st, s=ssq: p_unit_a(
                            bb, sb, t, q, s
                        )
                    yield lambda bb=bb, sb=sb, s=ssq: p_newton(bb, sb, s)
                    for t in range(NSB):
                        yield lambda bb=bb, sb=sb, t=t, q=qk_st, qs=qk16s: p_unit_b(
                            bb, sb, t, q, qs
                        )
                    for t in range(NSB):
                        yield lambda bb=bb, sb=sb, t=t, qs=qk16s: p_unit_c(
                            bb, sb, t, qs[t]
                        )

            # ---------------- A phase ----------------
            def emit_scores_exp(bb, half, h, lk, es_cur):
                j = bb * NTH + lk
                pss = pS.tile([128, 1024], F32, tag="sc", name="pss")
                for h2 in range(2):
                    nc.tensor.matmul(
                        pss[:, h2 * 512 : (h2 + 1) * 512],
                        lhsT=khT[
                            h * D : (h + 1) * D,
                            bb * L + lk * 128 : bb * L + (lk + 1) * 128,
                        ],
                        rhs=qhT[
                            h * D : (h + 1) * D,
                            bb * L + half * 1024 + h2 * 512 :
                            bb * L + half * 1024 + (h2 + 1) * 512,
                        ],
                        start=True,
                        stop=True,
                    )
                nc.scalar.activation(
                    out=es_cur[:, lk, :], in_=pss, func=AF.Exp,
                    scale=rr[:, j, 2 + h : 3 + h], bias=biasap[:, 0:1],
                )

            def emit_av_tile(bb, half, h, i, es_prev):
                """AV for lq-tile i (0..7 within half) of the PREVIOUS block."""
                po = pA.tile([128, 128], F32, tag="oacc", name="po")
                for lk in range(NLK):
                    nc.tensor.matmul(
                        po[:, 0 : D + 1],
                        lhsT=es_prev[:, lk, i * 128 : (i + 1) * 128],
                        rhs=vaug[:, bb * NTH + lk, h, :],
                        start=(lk == 0),
                        stop=(lk == NLK - 1),
                        skip_group_check=True,
                    )
                rd = nrm.tile([128, 1], F32, tag="rd")
                nc.vector.reciprocal(rd, po[:, D : D + 1])
                ti = half * 8 + i
                nc.vector.tensor_scalar(
                    out=o_both[:, bb, ti, h * D : (h + 1) * D],
                    in0=po[:, 0:D],
                    scalar1=rd[:, 0:1], scalar2=None, op0=ALU.mult,
                )

            # ---------------- O phase units ----------------
            def o_unit_t(bb, ti, box):
                """transpose o_both tile -> oT fp16 (PE + DVE copy)."""
                ps = pP.tile([128, 512], F32, tag="pp", name="ot_ps")
                tr = ps[:, 0:64].bitcast(F16)  # [128, 128]
                nc.tensor.transpose(tr, o_both[:, bb, ti, :], ident16)
                oT = obp.tile([128, 128], F16, tag="oT")
                nc.vector.tensor_copy(oT, tr)
                box.append(oT)

            def o_unit_mm(bb, ti, eo, box):
                """one out-proj half: matmul + evac + dma."""
                j = bb * NTH + ti
                pout = pO.tile([128, 512], F32, tag="po2", name="pout")
                nc.tensor.matmul(
                    pout, lhsT=box[0], rhs=wproj_sb[:, eo * 512 : (eo + 1) * 512],
                    start=True, stop=True,
                )
                ob = osb.tile([128, 512], F32, tag="ob")
                nc.vector.tensor_copy(ob, pout)
                nc.sync.dma_start(
                    out=outp[j * 128 : (j + 1) * 128, eo * 512 : (eo + 1) * 512],
                    in_=ob,
                )

            def gen_O(bb, half):
                for ti in range(half * 8, half * 8 + 8):
                    box = []
                    yield lambda bb=bb, ti=ti, box=box: o_unit_t(bb, ti, box)
                    for eo in range(2):
                        yield lambda bb=bb, ti=ti, eo=eo, box=box: o_unit_mm(
                            bb, ti, eo, box
                        )

            # ---------------- main schedule ----------------
            # Two pending queues: P units carry a deadline (start of the
            # first block of their batch); O units are deadline-free filler.
            pendP, pendO = [], []
            pidx, oidx = [0], [0]

            def drainP(n):
                k = 0
                while k < n and pidx[0] < len(pendP):
                    pendP[pidx[0]]()
                    pidx[0] += 1
                    k += 1
                return k

            def drainO(n):
                k = 0
                while k < n and oidx[0] < len(pendO):
                    pendO[oidx[0]]()
                    oidx[0] += 1
                    k += 1

            # prologue: first half of P(0); rest interleaves into block 0
            for u in gen_P(0, 0, NTH // NSB // 2):
                u()
            pendP.extend(gen_P(0, NTH // NSB // 2))
            pendP.extend(gen_P(1))

            blocks = [(bb, half, h) for bb in range(B) for half in range(2)
                      for h in range(HL)]
            prev_block = None
            prev_es = None
            o_ready = []          # (bb, half) chunks whose o_both is complete
            P_DEADLINE = 4 * NLK  # P(1) must be done by block 4 slot 0

            for bi, (bb, half, h) in enumerate(blocks):
                es_cur = esp.tile([128, NLK, 1024], F16, tag="es", name="es_all")
                for lk in range(NLK):
                    gslot = bi * NLK + lk
                    emit_scores_exp(bb, half, h, lk, es_cur)
                    # AV of the previous block: one lq-tile per 2 lk slots
                    if prev_block is not None and lk % 2 == 0:
                        pb, ph_alf, ph = prev_block
                        emit_av_tile(pb, ph_alf, ph, lk // 2, prev_es)
                        if lk == 14 and ph == 1:
                            o_ready.append((pb, ph_alf))
                    # deadline-paced drain of P work; O fills leftover slots
                    remP = len(pendP) - pidx[0]
                    if remP:
                        slots_left = max(1, P_DEADLINE - gslot)
                        need = -(-remP // slots_left)  # ceil
                        done = drainP(min(3, max(1, need)))
                    else:
                        done = 0
                    if done == 0:
                        drainO(1)
                # queue O work for completed halves
                while o_ready:
                    ob_b, ob_half = o_ready.pop(0)
                    pendO.extend(gen_O(ob_b, ob_half))
                prev_block = (bb, half, h)
                prev_es = es_cur

            # tail: AV of the last block + remaining O work
            pb, ph_alf, ph = prev_block
            for i in range(8):
                emit_av_tile(pb, ph_alf, ph, i, prev_es)
                drainO(2)
            o_ready.append((pb, ph_alf))
            while o_ready:
                ob_b, ob_half = o_ready.pop(0)
                pendO.extend(gen_O(ob_b, ob_half))
            drainP(len(pendP))
            drainO(len(pendO))

    nc.compile()
    return nc


_PERM = np.concatenate([np.arange(0, D, 2), np.arange(1, D, 2)])  # ev|od split


def _prep_inputs(x, pe, Wq, Wkv, Wproj, q_scale, k_scale):
    x = np.asarray(x, np.float32)
    xT = np.ascontiguousarray(x.reshape(BL, QD).T)                    # [QD, BL]
    xtt = np.ascontiguousarray(
        xT.reshape(CT, 128, NT, 128).transpose(2, 1, 0, 3)
    ).astype(np.float16)                                              # [NT, p, CT, n]

    pe = np.asarray(pe, np.float32)[0, 0]                             # [L, 32, 2, 2]
    qs, ks = np.asarray(q_scale, np.float32), np.asarray(k_scale, np.float32)

    def planes(scale):
        se, so = scale[0::2], scale[1::2]
        return (
            pe[:, :, 0, 0] * se[None, :],
            pe[:, :, 0, 1] * so[None, :],
            pe[:, :, 1, 0] * se[None, :],
            pe[:, :, 1, 1] * so[None, :],
        )

    pq, pk = planes(qs), planes(ks)
    coefs = np.empty((L, 4, 4, 32), np.float32)                       # [l, plane, grp, d2]
    for p_i in range(4):
        coefs[:, p_i, 0] = pq[p_i]
        coefs[:, p_i, 1] = pq[p_i]
        coefs[:, p_i, 2] = pk[p_i]
        coefs[:, p_i, 3] = pk[p_i]
    coefs = np.ascontiguousarray(coefs.reshape(NLK, 128, 4 * M)).astype(np.float16)

    Wq = np.asarray(Wq, np.float32)
    Wkv = np.asarray(Wkv, np.float32)
    Wproj = np.asarray(Wproj, np.float32)
    Wk_full, Wv_full = Wkv[:INNER], Wkv[INNER:]

    in_maps = []
    for c in range(NCORES):
        r0, r1 = c * M, (c + 1) * M
        wq_c, wk_c, wv_c = Wq[r0:r1], Wk_full[r0:r1], Wv_full[r0:r1]
        # permute q/k output dims to ev|od blocks per head (v stays in order)
        perm2 = np.concatenate([_PERM, _PERM + D])
        wq_c = wq_c[perm2]
        wk_c = wk_c[perm2]
        wqkv_c = np.concatenate([wq_c, wk_c, wv_c], axis=0)
        wqkv_t = np.ascontiguousarray(
            wqkv_c.T.reshape(CT, 128, 3 * M).transpose(1, 0, 2)
        ).astype(np.float16)                                          # [128, CT, 3M]
        wproj_c = np.ascontiguousarray(Wproj[:, r0:r1].T).astype(np.float16)
        in_maps.append(
            {"xt": xtt, "wqkv": wqkv_t, "wproj": wproj_c, "coefs": coefs}
        )
    return in_maps


def kernel(x, pe, Wq, Wkv, Wproj, bproj, q_scale, k_scale):
    if "nc" not in _CACHE:
        _CACHE["nc"] = _build_nc()
    nc = _CACHE["nc"]
    in_maps = _prep_inputs(x, pe, Wq, Wkv, Wproj, q_scale, k_scale)
    res = run_bass_kernel_spmd(nc, in_maps, core_ids=list(range(NCORES)))
    acc = np.zeros((BL, QD), np.float32)
    for c in range(NCORES):
        acc += np.asarray(res.results[c]["outp"], np.float32)
    acc += np.asarray(bproj, np.float32)[None, :]
    return acc.reshape(B, L, QD)
